# revision 6
# baseline (speedup 1.0000x reference)
"""MoE memory layer (attention + expert-choice MoE) on 8 Trainium2 NeuronCores.

Structure:
  Launch 1 (token-parallel): core c handles batch c//2, query-half c%2.
    LN1 -> QKV (transposed activation layouts) -> causal attention ->
    Wo + residual -> LN2 -> router MLP -> logits.
  Host: exact top-CAP token selection per expert via argpartition (the
    combine averages per-token, so only the selected SET matters), counts,
    dispatch gather.
  Launch 2 (expert-parallel): core e runs expert e's FFN over its CAP tokens,
    scatters outputs into a [N, D] buffer, ReduceScatter(add) combines across
    experts, each core emits its token shard of h + combined/counts.

Self-contained: hardcodes all shapes; no file reads.
"""
import sys

if "/opt/trn_rl_repo" not in sys.path:
    sys.path.insert(0, "/opt/trn_rl_repo")

import numpy as np
import concourse.bass as bass
import concourse.mybir as mybir
import concourse.tile as tile
from concourse.bass_utils import run_bass_kernel_spmd

P = 128
B, S, D = 4, 1024, 1024
H, HD = 16, 64
E = 8
N = B * S                  # 4096
CAP = 640
F = 4 * D                  # 4096
NC = 8
LN_EPS = 1e-5
DCH = D // P               # 8 chunks of the D axis
FCH = F // P               # 32 chunks of the F axis
QL = S // 2                # 512 query rows per core
TT = QL // P               # 4 token tiles per core shard
CT = CAP // P              # 5 dispatched-token tiles per expert
KV_T = S // P              # 8 kv tiles

f32 = mybir.dt.float32
i32 = mybir.dt.int32
AF = mybir.ActivationFunctionType
OP = mybir.AluOpType


def _split_excess_waits(nc):
    """This walrus build allows one sync wait per instruction; move extras onto
    EventSemaphore instructions inserted just before."""
    n = 0
    for fn in nc.m.functions:
        for bb in fn.blocks:
            new_insts = []
            for inst in bb.instructions:
                si = getattr(inst, "sync_info", None)
                if (not isinstance(inst, mybir.InstEventSemaphore) and si is not None
                        and si.on_wait is not None and len(si.on_wait) > 1):
                    waits = list(si.on_wait)
                    for w in waits[:-1]:
                        ev = mybir.InstEventSemaphore(
                            name=f"waitsplit_{n}", engine=inst.engine, ins=[], outs=[],
                            sync_info=mybir.SyncInfo(on_wait=[w], on_update=[]))
                        new_insts.append(ev)
                        n += 1
                    inst.sync_info = mybir.SyncInfo(on_wait=[waits[-1]],
                                                    on_update=si.on_update)
                new_insts.append(inst)
            bb.instructions[:] = new_insts
    return n


def _ln_stats_T(nc, pool, psp, ones_col, ones_row, x_view, nchunks, ntok, inv_d, tagp):
    """LN stats for a D-major activation view ([128, nchunks*ntok]; chunk j at
    cols [ntok*j, ...)).  Returns (rinvB, mrB): [128, ntok] tiles in `pool`
    holding rsqrt(var+eps) and mean*rinv, broadcast across partitions."""
    nsplits = [(o, min(512, ntok - o)) for o in range(0, ntok, 512)]

    musb = pool.tile([1, ntok], f32, tag=f"{tagp}_musb", name=f"{tagp}_musb")
    exsb = pool.tile([1, ntok], f32, tag=f"{tagp}_exsb", name=f"{tagp}_exsb")
    for (o, w) in nsplits:
        ps = psp.tile([1, w], f32, space="PSUM", tag=f"{tagp}_pmu", bufs=2, name=f"{tagp}_pmu")
        for j in range(nchunks):
            nc.tensor.matmul(out=ps[:], lhsT=ones_col[:],
                             rhs=x_view[:, ntok * j + o: ntok * j + o + w],
                             start=(j == 0), stop=(j == nchunks - 1))
        nc.vector.tensor_scalar(out=musb[:, o:o + w], in0=ps[:], scalar1=inv_d,
                                scalar2=None, op0=OP.mult)
        ps2 = psp.tile([1, w], f32, space="PSUM", tag=f"{tagp}_psq", bufs=2, name=f"{tagp}_psq")
        for j in range(nchunks):
            sq = pool.tile([P, w], f32, tag=f"{tagp}_sq", bufs=2, name=f"{tagp}_sq")
            nc.scalar.activation(sq[:], x_view[:, ntok * j + o: ntok * j + o + w], AF.Square)
            nc.tensor.matmul(out=ps2[:], lhsT=ones_col[:], rhs=sq[:],
                             start=(j == 0), stop=(j == nchunks - 1))
        nc.vector.tensor_scalar(out=exsb[:, o:o + w], in0=ps2[:], scalar1=inv_d,
                                scalar2=None, op0=OP.mult)
    # var = E[x^2] - mu^2 ; rinv = sqrt(1/(var+eps)) ; mr = mu*rinv
    var = pool.tile([1, ntok], f32, tag=f"{tagp}_var", name=f"{tagp}_var")
    nc.vector.tensor_tensor(out=var[:], in0=musb[:], in1=musb[:], op=OP.mult)
    nc.vector.tensor_tensor(out=var[:], in0=exsb[:], in1=var[:], op=OP.subtract)
    nc.vector.tensor_scalar(out=var[:], in0=var[:], scalar1=LN_EPS, scalar2=None, op0=OP.add)
    rinv = pool.tile([1, ntok], f32, tag=f"{tagp}_rinv", name=f"{tagp}_rinv")
    nc.vector.reciprocal(rinv[:], var[:])
    nc.scalar.activation(rinv[:], rinv[:], AF.Sqrt)
    mr = pool.tile([1, ntok], f32, tag=f"{tagp}_mr", name=f"{tagp}_mr")
    nc.vector.tensor_tensor(out=mr[:], in0=musb[:], in1=rinv[:], op=OP.mult)
    rinvB = pool.tile([P, ntok], f32, tag=f"{tagp}_rinvB", name=f"{tagp}_rinvB")
    mrB = pool.tile([P, ntok], f32, tag=f"{tagp}_mrB", name=f"{tagp}_mrB")
    for (o, w) in nsplits:
        for src, dst in ((rinv, rinvB), (mr, mrB)):
            ps = psp.tile([P, w], f32, space="PSUM", tag=f"{tagp}_pbc", bufs=2, name=f"{tagp}_pbc")
            nc.tensor.matmul(out=ps[:], lhsT=ones_row[:1, :], rhs=src[:, o:o + w],
                             start=True, stop=True)
            nc.scalar.copy(out=dst[:, o:o + w], in_=ps[:])
    return rinvB, mrB


def _ln_apply_T(nc, out_view, x_view, rinvB, mrB, gv, bv, nchunks, ntok):
    """out = (x - mu) * rinv * g + b in D-major layout; g/b are [128, nchunks]."""
    for j in range(nchunks):
        sl = slice(ntok * j, ntok * (j + 1))
        nc.vector.tensor_tensor(out=out_view[:, sl], in0=x_view[:, sl], in1=rinvB[:], op=OP.mult)
        nc.vector.tensor_tensor(out=out_view[:, sl], in0=out_view[:, sl], in1=mrB[:], op=OP.subtract)
        nc.vector.tensor_scalar(out=out_view[:, sl], in0=out_view[:, sl],
                                scalar1=gv[:, j:j + 1], scalar2=bv[:, j:j + 1],
                                op0=OP.mult, op1=OP.add)


def build_launch1():
    nc = bass.Bass()
    xbT = nc.declare_dram_parameter("xbT", [D, S], f32, isOutput=False)
    mask = nc.declare_dram_parameter("mask", [S, QL], f32, isOutput=False)
    Wq = nc.declare_dram_parameter("Wq", [D, D], f32, isOutput=False)
    Wk = nc.declare_dram_parameter("Wk", [D, D], f32, isOutput=False)
    Wv = nc.declare_dram_parameter("Wv", [D, D], f32, isOutput=False)
    Wo = nc.declare_dram_parameter("Wo", [D, D], f32, isOutput=False)
    Wr1 = nc.declare_dram_parameter("Wr1", [D, F], f32, isOutput=False)
    wr2v = nc.declare_dram_parameter("wr2v", [P, FCH * E], f32, isOutput=False)
    br1v = nc.declare_dram_parameter("br1v", [P, FCH], f32, isOutput=False)
    br2B = nc.declare_dram_parameter("br2B", [P, E], f32, isOutput=False)
    g1v = nc.declare_dram_parameter("g1v", [P, DCH], f32, isOutput=False)
    b1v = nc.declare_dram_parameter("b1v", [P, DCH], f32, isOutput=False)
    g2v = nc.declare_dram_parameter("g2v", [P, DCH], f32, isOutput=False)
    b2v = nc.declare_dram_parameter("b2v", [P, DCH], f32, isOutput=False)
    hT_out = nc.declare_dram_parameter("hT_out", [D, QL], f32, isOutput=True)
    logits_out = nc.declare_dram_parameter("logits_out", [QL, E], f32, isOutput=True)
    impsum_out = nc.declare_dram_parameter("impsum_out", [1, E], f32, isOutput=True)

    with tile.TileContext(nc) as tc:
        with tc.tile_pool(name="big", bufs=1) as big, \
             tc.tile_pool(name="st", bufs=1) as st, \
             tc.tile_pool(name="dram", bufs=1, space="DRAM") as dram:

            KTd = dram.tile([D, S], f32)    # [d, kv]
            Vd = dram.tile([S, D], f32)     # [kv, d]
            QTd = dram.tile([D, QL], f32)   # [d, q]

            ones_col = st.tile([P, 1], f32, tag="ones_col")
            nc.vector.memset(ones_col[:], 1.0)
            ones_row = st.tile([1, P], f32, tag="ones_row")
            nc.vector.memset(ones_row[:], 1.0)
            g1s = st.tile([P, DCH], f32, tag="g1s")
            nc.sync.dma_start(out=g1s[:], in_=g1v[:])
            b1s = st.tile([P, DCH], f32, tag="b1s")
            nc.sync.dma_start(out=b1s[:], in_=b1v[:])
            g2s = st.tile([P, DCH], f32, tag="g2s")
            nc.sync.dma_start(out=g2s[:], in_=g2v[:])
            b2s = st.tile([P, DCH], f32, tag="b2s")
            nc.sync.dma_start(out=b2s[:], in_=b2v[:])

            # persistent across stages: q-half residual, attention out, h, hn
            xqT_sb = big.tile([P, DCH * QL], f32, tag="xqT")
            OT_sb = big.tile([P, DCH * QL], f32, tag="OT")
            hT_sb = big.tile([P, DCH * QL], f32, tag="hT")
            hnT_sb = big.tile([P, DCH * QL], f32, tag="hnT")

            with tc.tile_pool(name="pAB", bufs=1) as pAB:
                xnT_sb = pAB.tile([P, DCH * S], f32, tag="xnT")

                # ---- A: LN1 ----
                with tc.tile_pool(name="pA", bufs=1) as pA, \
                     tc.tile_pool(name="psA", bufs=1, space="PSUM") as psp:
                    xbT_sb = pA.tile([P, DCH * S], f32, tag="xbT")
                    nc.sync.dma_start(out=xbT_sb[:].rearrange("p (j s) -> p j s", s=S),
                                      in_=xbT.rearrange("(j p) s -> p j s", p=P))
                    rinvB, mrB = _ln_stats_T(nc, pA, psp, ones_col, ones_row,
                                             xbT_sb[:], DCH, S, 1.0 / D, "ln1")
                    _ln_apply_T(nc, xnT_sb[:], xbT_sb[:], rinvB, mrB, g1s, b1s, DCH, S)
                    for j in range(DCH):
                        nc.vector.tensor_copy(out=xqT_sb[:, QL * j: QL * (j + 1)],
                                              in_=xbT_sb[:, S * j: S * j + QL])

                # ---- B: KT, V, QT -> DRAM ----
                with tc.tile_pool(name="pB", bufs=1) as pB, \
                     tc.tile_pool(name="psB", bufs=1, space="PSUM") as psp:
                    for i in range(DCH):
                        wkb = pB.tile([P, DCH * P], f32, tag="wkb", bufs=3, name="wkb")
                        nc.sync.dma_start(out=wkb[:].rearrange("p (j m) -> p j m", m=P),
                                          in_=Wk.rearrange("(j p) m -> p j m", p=P)[:, :, P * i:P * (i + 1)])
                        wkb_v = wkb[:].rearrange("p (j m) -> p j m", m=P)
                        for o in (0, 512):
                            ps = psp.tile([P, 512], f32, space="PSUM", tag="pk", bufs=2, name="pk")
                            for j in range(DCH):
                                nc.tensor.matmul(out=ps[:], lhsT=wkb_v[:, j, :],
                                                 rhs=xnT_sb[:, S * j + o: S * j + o + 512],
                                                 start=(j == 0), stop=(j == DCH - 1))
                            ev = pB.tile([P, 512], f32, tag="evk", bufs=3, name="evk")
                            nc.scalar.copy(out=ev[:], in_=ps[:])
                            nc.sync.dma_start(out=KTd[P * i: P * (i + 1), o:o + 512], in_=ev[:])
                    for o in (0, 512):
                        wvh = pB.tile([P, DCH * 512], f32, tag="wvh", bufs=2, name="wvh")
                        nc.sync.dma_start(out=wvh[:].rearrange("p (j n) -> p j n", n=512),
                                          in_=Wv.rearrange("(j p) n -> p j n", p=P)[:, :, o:o + 512])
                        wvh_v = wvh[:].rearrange("p (j n) -> p j n", n=512)
                        for i in range(DCH):
                            ps = psp.tile([P, 512], f32, space="PSUM", tag="pv", bufs=2, name="pv")
                            for j in range(DCH):
                                nc.tensor.matmul(out=ps[:],
                                                 lhsT=xnT_sb[:, S * j + P * i: S * j + P * (i + 1)],
                                                 rhs=wvh_v[:, j, :],
                                                 start=(j == 0), stop=(j == DCH - 1))
                            ev = pB.tile([P, 512], f32, tag="evv", bufs=3, name="evv")
                            nc.scalar.copy(out=ev[:], in_=ps[:])
                            nc.sync.dma_start(out=Vd[P * i: P * (i + 1), o:o + 512], in_=ev[:])
                    for i in range(DCH):
                        wqb = pB.tile([P, DCH * P], f32, tag="wqb", bufs=3, name="wqb")
                        nc.sync.dma_start(out=wqb[:].rearrange("p (j m) -> p j m", m=P),
                                          in_=Wq.rearrange("(j p) m -> p j m", p=P)[:, :, P * i:P * (i + 1)])
                        wqb_v = wqb[:].rearrange("p (j m) -> p j m", m=P)
                        ps = psp.tile([P, QL], f32, space="PSUM", tag="pq", bufs=2, name="pq")
                        for j in range(DCH):
                            nc.tensor.matmul(out=ps[:], lhsT=wqb_v[:, j, :],
                                             rhs=xnT_sb[:, S * j: S * j + QL],
                                             start=(j == 0), stop=(j == DCH - 1))
                        ev = pB.tile([P, QL], f32, tag="evq", bufs=3, name="evq")
                        nc.scalar.copy(out=ev[:], in_=ps[:])
                        nc.sync.dma_start(out=QTd[P * i: P * (i + 1), :], in_=ev[:])

            # ---- C: attention (scores transposed: [kv, q]) ----
            with tc.tile_pool(name="pC", bufs=1) as pC, \
                 tc.tile_pool(name="psC", bufs=1, space="PSUM") as psp:
                mask_sb = pC.tile([P, KV_T * QL], f32, tag="mask")
                nc.sync.dma_start(out=mask_sb[:].rearrange("p (j q) -> p j q", q=QL),
                                  in_=mask.rearrange("(j p) q -> p j q", p=P))
                for h in range(H):
                    jh, off = h // 2, HD * (h % 2)
                    kth = pC.tile([HD, S], f32, tag="kth", bufs=2, name="kth")
                    nc.sync.dma_start(out=kth[:], in_=KTd[HD * h: HD * (h + 1), :])
                    qth = pC.tile([HD, QL], f32, tag="qth", bufs=2, name="qth")
                    nc.sync.dma_start(out=qth[:], in_=QTd[HD * h: HD * (h + 1), :])
                    vh = pC.tile([P, KV_T * HD], f32, tag="vh", bufs=2, name="vh")
                    nc.sync.dma_start(out=vh[:].rearrange("p (j d) -> p j d", d=HD),
                                      in_=Vd.rearrange("(j p) d -> p j d", p=P)[:, :, HD * h: HD * (h + 1)])
                    pts = []
                    for kt in range(KV_T):
                        ps = psp.tile([P, QL], f32, space="PSUM", tag="pscore", bufs=2, name="pscore")
                        nc.tensor.matmul(out=ps[:], lhsT=kth[:, P * kt: P * (kt + 1)],
                                         rhs=qth[:], start=True, stop=True)
                        pt = pC.tile([P, QL], f32, tag=f"pt{kt}", bufs=1, name=f"pt{kt}")
                        nc.scalar.activation(pt[:], ps[:], AF.Exp, scale=0.125)
                        nc.vector.tensor_tensor(out=pt[:], in0=pt[:],
                                                in1=mask_sb[:, QL * kt: QL * (kt + 1)], op=OP.mult)
                        pts.append(pt)
                    pssum = psp.tile([1, QL], f32, space="PSUM", tag="pssum", bufs=2, name="pssum")
                    for kt in range(KV_T):
                        nc.tensor.matmul(out=pssum[:], lhsT=ones_col[:], rhs=pts[kt][:],
                                         start=(kt == 0), stop=(kt == KV_T - 1))
                    rec = pC.tile([1, QL], f32, tag="rec", bufs=2, name="rec")
                    nc.vector.reciprocal(rec[:], pssum[:])
                    psb = psp.tile([64, QL], f32, space="PSUM", tag="psb", bufs=2, name="psb")
                    nc.tensor.matmul(out=psb[:], lhsT=ones_row[:1, :64], rhs=rec[:],
                                     start=True, stop=True)
                    recB = pC.tile([64, QL], f32, tag="recB", bufs=2, name="recB")
                    nc.scalar.copy(out=recB[:], in_=psb[:])
                    pso = psp.tile([64, QL], f32, space="PSUM", tag="pso", bufs=2, name="pso")
                    for kt in range(KV_T):
                        nc.tensor.matmul(out=pso[:], lhsT=vh[:, HD * kt: HD * (kt + 1)],
                                         rhs=pts[kt][:], start=(kt == 0), stop=(kt == KV_T - 1))
                    nc.vector.tensor_tensor(out=OT_sb[off:off + HD, QL * jh: QL * (jh + 1)],
                                            in0=pso[:], in1=recB[:], op=OP.mult)

            # ---- D: hT = Wo.T-block @ OT + x residual ----
            with tc.tile_pool(name="pD", bufs=1) as pD, \
                 tc.tile_pool(name="psD", bufs=1, space="PSUM") as psp:
                for i in range(DCH):
                    wob = pD.tile([P, DCH * P], f32, tag="wob", bufs=3, name="wob")
                    nc.sync.dma_start(out=wob[:].rearrange("p (j m) -> p j m", m=P),
                                      in_=Wo.rearrange("(j p) m -> p j m", p=P)[:, :, P * i:P * (i + 1)])
                    wob_v = wob[:].rearrange("p (j m) -> p j m", m=P)
                    ps = psp.tile([P, QL], f32, space="PSUM", tag="ph", bufs=2, name="ph")
                    for j in range(DCH):
                        nc.tensor.matmul(out=ps[:], lhsT=wob_v[:, j, :],
                                         rhs=OT_sb[:, QL * j: QL * (j + 1)],
                                         start=(j == 0), stop=(j == DCH - 1))
                    nc.vector.tensor_tensor(out=hT_sb[:, QL * i: QL * (i + 1)], in0=ps[:],
                                            in1=xqT_sb[:, QL * i: QL * (i + 1)], op=OP.add)
                    nc.sync.dma_start(out=hT_out[P * i: P * (i + 1), :],
                                      in_=hT_sb[:, QL * i: QL * (i + 1)])

            # ---- E: LN2 -> hnT ----
            with tc.tile_pool(name="pE", bufs=1) as pE, \
                 tc.tile_pool(name="psE", bufs=1, space="PSUM") as psp:
                rinv2B, mr2B = _ln_stats_T(nc, pE, psp, ones_col, ones_row,
                                           hT_sb[:], DCH, QL, 1.0 / D, "ln2")
                _ln_apply_T(nc, hnT_sb[:], hT_sb[:], rinv2B, mr2B, g2s, b2s, DCH, QL)

            # ---- F+G: router ----
            with tc.tile_pool(name="pFG", bufs=1) as pFG, \
                 tc.tile_pool(name="psFG", bufs=1, space="PSUM") as psp:
                br1s = pFG.tile([P, FCH], f32, tag="br1s")
                nc.sync.dma_start(out=br1s[:], in_=br1v[:])
                r1T_sb = pFG.tile([P, FCH * QL], f32, tag="r1T")
                for i in range(FCH):
                    w1b = pFG.tile([P, DCH * P], f32, tag="w1b", bufs=3, name="w1b")
                    nc.sync.dma_start(out=w1b[:].rearrange("p (j m) -> p j m", m=P),
                                      in_=Wr1.rearrange("(j p) m -> p j m", p=P)[:, :, P * i:P * (i + 1)])
                    w1b_v = w1b[:].rearrange("p (j m) -> p j m", m=P)
                    ps = psp.tile([P, QL], f32, space="PSUM", tag="pr", bufs=2, name="pr")
                    for j in range(DCH):
                        nc.tensor.matmul(out=ps[:], lhsT=w1b_v[:, j, :],
                                         rhs=hnT_sb[:, QL * j: QL * (j + 1)],
                                         start=(j == 0), stop=(j == DCH - 1))
                    nc.scalar.activation(r1T_sb[:, QL * i: QL * (i + 1)], ps[:], AF.Relu,
                                         bias=br1s[:, i:i + 1])

                wr2s = pFG.tile([P, FCH * E], f32, tag="wr2s")
                nc.sync.dma_start(out=wr2s[:], in_=wr2v[:])
                br2s = pFG.tile([P, E], f32, tag="br2s")
                nc.sync.dma_start(out=br2s[:], in_=br2B[:])
                prs = []
                for mt in range(TT):
                    ps = psp.tile([P, E], f32, space="PSUM", tag="pl", bufs=2, name="pl")
                    for j in range(FCH):
                        nc.tensor.matmul(out=ps[:],
                                         lhsT=r1T_sb[:, QL * j + P * mt: QL * j + P * (mt + 1)],
                                         rhs=wr2s[:, E * j: E * (j + 1)],
                                         start=(j == 0), stop=(j == FCH - 1))
                    lg = pFG.tile([P, E], f32, tag=f"lg{mt}", name=f"lg{mt}")
                    nc.vector.tensor_tensor(out=lg[:], in0=ps[:], in1=br2s[:], op=OP.add)
                    nc.sync.dma_start(out=logits_out[P * mt: P * (mt + 1), :], in_=lg[:])
                    pr = pFG.tile([P, E], f32, tag=f"pr{mt}", name=f"pr{mt}")
                    racc = pFG.tile([P, 1], f32, tag=f"racc{mt}", name=f"racc{mt}")
                    nc.scalar.activation(pr[:], lg[:], AF.Exp, accum_out=racc[:])
                    rrec = pFG.tile([P, 1], f32, tag=f"rrec{mt}", name=f"rrec{mt}")
                    nc.vector.reciprocal(rrec[:], racc[:])
                    nc.vector.tensor_scalar(out=pr[:], in0=pr[:], scalar1=rrec[:, :1],
                                            scalar2=None, op0=OP.mult)
                    prs.append(pr)
                psi = psp.tile([1, E], f32, space="PSUM", tag="psi", bufs=1, name="psi")
                for mt in range(TT):
                    nc.tensor.matmul(out=psi[:], lhsT=ones_col[:], rhs=prs[mt][:],
                                     start=(mt == 0), stop=(mt == TT - 1))
                imp = pFG.tile([1, E], f32, tag="imp", name="imp")
                nc.scalar.copy(out=imp[:], in_=psi[:])
                nc.sync.dma_start(out=impsum_out[:], in_=imp[:])

    _split_excess_waits(nc)
    return nc


def build_launch2():
    nc = bass.Bass()
    hdT = nc.declare_dram_parameter("hdT", [D, CAP], f32, isOutput=False)
    We1 = nc.declare_dram_parameter("We1", [D, F], f32, isOutput=False)
    We2 = nc.declare_dram_parameter("We2", [F, D], f32, isOutput=False)
    be1v = nc.declare_dram_parameter("be1v", [P, FCH], f32, isOutput=False)
    be2B = nc.declare_dram_parameter("be2B", [P, D], f32, isOutput=False)
    g2v = nc.declare_dram_parameter("g2v", [P, DCH], f32, isOutput=False)
    b2v = nc.declare_dram_parameter("b2v", [P, DCH], f32, isOutput=False)
    scat_idx = nc.declare_dram_parameter("scat_idx", [P, CT], i32, isOutput=False)
    inv_cnt = nc.declare_dram_parameter("inv_cnt", [P, TT], f32, isOutput=False)
    h_own = nc.declare_dram_parameter("h_own", [QL, D], f32, isOutput=False)
    final_out = nc.declare_dram_parameter("final_out", [QL, D], f32, isOutput=True)

    with tile.TileContext(nc) as tc:
        with tc.tile_pool(name="st", bufs=1) as st, \
             tc.tile_pool(name="dram", bufs=1, space="DRAM") as dram:

            zbuf = dram.tile([N, D], f32)
            rs_out = dram.tile([QL, D], f32)

            ones_col = st.tile([P, 1], f32, tag="ones_col")
            nc.vector.memset(ones_col[:], 1.0)
            ones_row = st.tile([1, P], f32, tag="ones_row")
            nc.vector.memset(ones_row[:], 1.0)
            g2s = st.tile([P, DCH], f32, tag="g2s")
            nc.sync.dma_start(out=g2s[:], in_=g2v[:])
            b2s = st.tile([P, DCH], f32, tag="b2s")
            nc.sync.dma_start(out=b2s[:], in_=b2v[:])
            be1s = st.tile([P, FCH], f32, tag="be1s")
            nc.sync.dma_start(out=be1s[:], in_=be1v[:])
            be2s = st.tile([P, D], f32, tag="be2s")
            nc.sync.dma_start(out=be2s[:], in_=be2B[:])

            # zero the scatter buffer early (overlaps with compute)
            zt = st.tile([P, D], f32, tag="zt")
            nc.vector.memset(zt[:], 0.0)
            for i in range(N // P):
                nc.sync.dma_start(out=zbuf[P * i: P * (i + 1), :], in_=zt[:])

            eout_sb = st.tile([P, CT * D], f32, tag="eout")   # token tile t at cols [D*t ...)

            with tc.tile_pool(name="pBC", bufs=1) as pBC:
                hidT_sb = pBC.tile([P, FCH * CAP], f32, tag="hidT")

                # ---- A: LN over dispatched tokens (D-major) ----
                with tc.tile_pool(name="pA", bufs=1) as pA:
                    hdT_sb = pA.tile([P, DCH * CAP], f32, tag="hdT")
                    nc.sync.dma_start(out=hdT_sb[:].rearrange("p (j c) -> p j c", c=CAP),
                                      in_=hdT.rearrange("(j p) c -> p j c", p=P))
                    with tc.tile_pool(name="psA2", bufs=1, space="PSUM") as psp:
                        rinvB, mrB = _ln_stats_T(nc, pA, psp, ones_col, ones_row,
                                                 hdT_sb[:], DCH, CAP, 1.0 / D, "lnd")
                    xdT_sb = pA.tile([P, DCH * CAP], f32, tag="xdT")
                    _ln_apply_T(nc, xdT_sb[:], hdT_sb[:], rinvB, mrB, g2s, b2s, DCH, CAP)

                    # ---- B: hidT = relu(We1.T-block @ xdT + be1) ----
                    psp_cm = tc.tile_pool(name="psB2", bufs=1, space="PSUM")
                    psp = psp_cm.__enter__()
                    for i in range(FCH):
                        w1b = pA.tile([P, DCH * P], f32, tag="w1b", bufs=3, name="w1b")
                        nc.sync.dma_start(out=w1b[:].rearrange("p (j m) -> p j m", m=P),
                                          in_=We1.rearrange("(j p) m -> p j m", p=P)[:, :, P * i:P * (i + 1)])
                        w1b_v = w1b[:].rearrange("p (j m) -> p j m", m=P)
                        for (o, w) in ((0, 512), (512, 128)):
                            ps = psp.tile([P, w], f32, space="PSUM", tag=f"pb{o}", bufs=2, name=f"pb{o}")
                            for j in range(DCH):
                                nc.tensor.matmul(out=ps[:], lhsT=w1b_v[:, j, :],
                                                 rhs=xdT_sb[:, CAP * j + o: CAP * j + o + w],
                                                 start=(j == 0), stop=(j == DCH - 1))
                            nc.scalar.activation(hidT_sb[:, CAP * i + o: CAP * i + o + w], ps[:],
                                                 AF.Relu, bias=be1s[:, i:i + 1])

                    psp_cm.__exit__(None, None, None)

                # ---- C: eout (token-major) = hidT.T-block @ We2 + be2 ----
                with tc.tile_pool(name="pC", bufs=1) as pC, \
                     tc.tile_pool(name="psC2", bufs=1, space="PSUM") as psp:
                    for o in (0, 512):
                        pes = [psp.tile([P, 512], f32, space="PSUM", tag=f"pe{t}", bufs=1,
                                        name=f"pe{t}") for t in range(CT)]
                        for j in range(FCH):
                            w2b = pC.tile([P, 512], f32, tag="w2b", bufs=3, name="w2b")
                            nc.sync.dma_start(out=w2b[:], in_=We2[P * j: P * (j + 1), o:o + 512])
                            for t in range(CT):
                                nc.tensor.matmul(out=pes[t][:],
                                                 lhsT=hidT_sb[:, CAP * j + P * t: CAP * j + P * (t + 1)],
                                                 rhs=w2b[:], start=(j == 0), stop=(j == FCH - 1))
                        for t in range(CT):
                            nc.vector.tensor_tensor(out=eout_sb[:, D * t + o: D * t + o + 512],
                                                    in0=pes[t][:], in1=be2s[:, o:o + 512], op=OP.add)

            # ---- D: scatter rows into zbuf by token index ----
            idx_sb = st.tile([P, CT], i32, tag="idx_sb")
            nc.sync.dma_start(out=idx_sb[:], in_=scat_idx[:])
            for t in range(CT):
                nc.gpsimd.indirect_dma_start(
                    out=zbuf[:],
                    out_offset=bass.IndirectOffsetOnAxis(ap=idx_sb[:, t:t + 1], axis=0),
                    in_=eout_sb[:, D * t: D * (t + 1)], in_offset=None)

            # ---- E: combine across experts ----
            nc.gpsimd.collective_compute(
                "ReduceScatter", OP.add, ins=[zbuf.opt()], outs=[rs_out.opt()],
                replica_groups=[list(range(NC))])

            # ---- F: final = h_own + rs_out * inv_cnt ----
            with tc.tile_pool(name="pF", bufs=1) as pF:
                ho_sb = pF.tile([P, TT * D], f32, tag="ho")
                nc.sync.dma_start(out=ho_sb[:].rearrange("p (t d) -> p t d", d=D),
                                  in_=h_own.rearrange("(t p) d -> p t d", p=P))
                icv = pF.tile([P, TT], f32, tag="icv")
                nc.sync.dma_start(out=icv[:], in_=inv_cnt[:])
                for t in range(TT):
                    rt = pF.tile([P, D], f32, tag="rt", bufs=2, name="rt")
                    nc.sync.dma_start(out=rt[:], in_=rs_out[P * t: P * (t + 1), :])
                    ft = pF.tile([P, D], f32, tag="ft", bufs=2, name="ft")
                    nc.vector.tensor_scalar(out=ft[:], in0=rt[:], scalar1=icv[:, t:t + 1],
                                            scalar2=None, op0=OP.mult)
                    nc.vector.tensor_tensor(out=ft[:], in0=ft[:],
                                            in1=ho_sb[:, D * t: D * (t + 1)], op=OP.add)
                    nc.sync.dma_start(out=final_out[P * t: P * (t + 1), :], in_=ft[:])

    _split_excess_waits(nc)
    return nc


_KERNELS = {}
_last_in_maps1 = None
_last_in_maps2 = None


def _get_kernel(which):
    if which not in _KERNELS:
        _KERNELS[which] = build_launch1() if which == 1 else build_launch2()
    return _KERNELS[which]


def _chunked(vec, nch):
    """[nch*128] -> [128, nch] with column j = vec[128j:128j+128]."""
    return np.ascontiguousarray(vec.reshape(nch, P).T)


def kernel(x, g1, b1, g2, b2, Wq, Wk, Wv, Wo, Wr1, br1, Wr2, br2,
           We1, be1, We2, be2):
    x = np.asarray(x, dtype=np.float32)
    g1, b1, g2, b2 = (np.asarray(v, np.float32) for v in (g1, b1, g2, b2))
    Wq, Wk, Wv, Wo = (np.ascontiguousarray(np.asarray(v, np.float32)) for v in (Wq, Wk, Wv, Wo))
    Wr1 = np.ascontiguousarray(np.asarray(Wr1, np.float32))
    br1 = np.asarray(br1, np.float32)
    Wr2 = np.asarray(Wr2, np.float32)
    br2 = np.asarray(br2, np.float32)
    We1 = np.asarray(We1, np.float32)
    be1 = np.asarray(be1, np.float32)
    We2 = np.asarray(We2, np.float32)
    be2 = np.asarray(be2, np.float32)

    # ---- launch 1 inputs ----
    wr2v = np.ascontiguousarray(Wr2.reshape(FCH, P, E).transpose(1, 0, 2).reshape(P, FCH * E))
    br1v = _chunked(br1, FCH)
    br2B = np.ascontiguousarray(np.broadcast_to(br2[None, :], (P, E)))
    g1v, b1v, g2v, b2v = (_chunked(v, DCH) for v in (g1, b1, g2, b2))

    r_idx = np.arange(S)
    mask_even = (r_idx[:, None] <= np.arange(QL)[None, :]).astype(np.float32)
    mask_odd = mask_even.copy()
    mask_odd[QL:, :] = 1.0

    in_maps1 = []
    for c in range(NC):
        b, qh = c // 2, c % 2
        xr = np.roll(x[b], -QL * qh, axis=0) if qh else x[b]
        in_maps1.append({
            "xbT": np.ascontiguousarray(xr.T),
            "mask": mask_odd if qh else mask_even,
            "Wq": Wq, "Wk": Wk, "Wv": Wv, "Wo": Wo, "Wr1": Wr1,
            "wr2v": wr2v, "br1v": br1v, "br2B": br2B,
            "g1v": g1v, "b1v": b1v, "g2v": g2v, "b2v": b2v,
        })
    global _last_in_maps1
    _last_in_maps1 = in_maps1
    res1 = run_bass_kernel_spmd(_get_kernel(1), in_maps1, list(range(NC)))

    h_flat = np.concatenate([np.ascontiguousarray(r["hT_out"].T) for r in res1.results], 0)
    logits = np.concatenate([r["logits_out"] for r in res1.results], 0)
    impsum = np.sum([r["impsum_out"][0] for r in res1.results], axis=0)

    # ---- routing on host (exact top-CAP set per expert) ----
    idx = np.argpartition(-logits.T, CAP - 1, axis=1)[:, :CAP].astype(np.int32)  # [E, CAP]
    cnt = np.bincount(idx.ravel(), minlength=N).astype(np.float32)
    inv_cnt_full = (1.0 / np.maximum(cnt, 1.0)).astype(np.float32)

    in_maps2 = []
    for e in range(NC):
        hd = h_flat[idx[e]]                       # [CAP, D]
        in_maps2.append({
            "hdT": np.ascontiguousarray(hd.T),
            "We1": np.ascontiguousarray(We1[e]),
            "We2": np.ascontiguousarray(We2[e]),
            "be1v": _chunked(be1[e], FCH),
            "be2B": np.ascontiguousarray(np.broadcast_to(be2[e][None, :], (P, D))),
            "g2v": g2v, "b2v": b2v,
            "scat_idx": np.ascontiguousarray(idx[e].reshape(CT, P).T),
            "inv_cnt": _chunked(inv_cnt_full[QL * e: QL * (e + 1)], TT),
            "h_own": np.ascontiguousarray(h_flat[QL * e: QL * (e + 1)]),
        })
    global _last_in_maps2
    _last_in_maps2 = in_maps2
    res2 = run_bass_kernel_spmd(_get_kernel(2), in_maps2, list(range(NC)))

    out_flat = np.concatenate([r["final_out"] for r in res2.results], 0)
    h_final = np.ascontiguousarray(out_flat.reshape(B, S, D))

    imp = impsum.astype(np.float64)
    imp_loss = np.float32(imp.var(ddof=1) / (imp.mean() ** 2 + 1e-6))
    load_loss = np.float32(0.0)
    return h_final, load_loss, imp_loss
